# revision 1
# baseline (speedup 1.0000x reference)
# Trainium2 Bass kernel for nn_Encoder_81509889343552.
# Encoder-decoder CNN (7x7 conv -> 4 stride-2 convs -> 4 stride-2 convT -> 7x7
# conv + InstanceNorm/ReLU, tanh) followed by a masked segment mean.
#
# Sharding: batch-parallel SPMD over 8 cores (B=4 images; cores 4-7 duplicate
# images 0-3; their outputs are discarded). Convs run as float32r matmuls
# (FP22-truncated fp32 at full PE rate for N>=256) over 128-channel blocks,
# with taps accumulated in PSUM via shifted/strided access patterns.
# InstanceNorm: per-channel stats via bn_stats on PSUM tiles during
# evacuation; relu((x-m)*rsqrt(v+eps)) is fused into the consumer layer's
# on-load activation (per-partition scale/bias). Conv biases before an
# InstanceNorm are mathematically no-ops and dropped; only the final bias
# (bf) is applied.
import sys

sys.path.insert(0, "/opt/trn_rl_repo")

import contextlib

import numpy as np

import concourse.bass as bass
import concourse.bacc as bacc
import concourse.tile as tile
from concourse import mybir
from concourse.bass_utils import run_bass_kernel_spmd

F32 = mybir.dt.float32
F32R = mybir.dt.float32r
I32 = mybir.dt.int32
AF = mybir.ActivationFunctionType
ALU = mybir.AluOpType

B, H, W = 4, 512, 512
BASE, NL = 64, 4
EPS = 1e-5
P = 128


def _ap(base, extra_off, dims):
    """AP sharing `base`'s tensor/partition dim, extra elem offset, free dims."""
    return bass.AP(
        tensor=base.tensor,
        offset=base.offset + extra_off,
        ap=[list(base.ap[0])] + [list(d) for d in dims],
    )


def _dap(handle, off, dims):
    """DRAM AP from a tensor handle with explicit dims (first = partition)."""
    return bass.AP(tensor=handle, offset=off, ap=[list(d) for d in dims])


# ----------------------------------------------------------------------------
# Host-side weight preprocessing
# ----------------------------------------------------------------------------

def prep_weights(inp):
    w = {}
    # layer 1: K-partition p = ci*16 + dy*2 + l (dy 0..7, l 0..1); M = r*64+co.
    # Matmul d covers kx = 2d+l; lhsT[p, r*64+co] = w0[co, ci, dy-r, 2d+l].
    w0 = np.asarray(inp["w0"], np.float32)  # [64, 3, 7, 7]
    w1 = np.zeros((4, 48, 128), np.float32)
    for d in range(4):
        for l in range(2):
            kx = 2 * d + l
            if kx > 6:
                continue
            for r in range(2):
                for ky in range(7):
                    dy = ky + r
                    for ci in range(3):
                        w1[d, ci * 16 + dy * 2 + l, r * 64:(r + 1) * 64] = \
                            w0[:, ci, ky, kx]
    w["w1"] = w1

    # down convs: blob [cbo, cbi, 3, 3, K, 128]
    c = BASE
    for i in range(NL):
        dw = np.asarray(inp[f"dw{i}"], np.float32)  # [2c, c, 3, 3]
        cbo, cbi, K = (2 * c) // P, max(c // P, 1), min(c, P)
        blob = np.zeros((cbo, cbi, 3, 3, K, P), np.float32)
        for m in range(cbo):
            for cb in range(cbi):
                for dy in range(3):
                    for dx in range(3):
                        blob[m, cb, dy, dx] = dw[m * P:(m + 1) * P,
                                                 cb * K:(cb + 1) * K, dy, dx].T
        w[f"wd{i}"] = blob
        c *= 2

    # up convs (torch convT layout uw [Cin, Cout, 3, 3]):
    # out[Y, X] += uw[ci, co, ky, kx] * in[i, j], Y = 2i - 1 + ky, X = 2j - 1 + kx
    for i in range(NL):
        uw = np.asarray(inp[f"uw{i}"], np.float32)
        Cin_, Cout_ = uw.shape[0], uw.shape[1]
        cbi, cbo, Mo = Cin_ // P, max(Cout_ // P, 1), min(Cout_, P)
        blob = np.zeros((cbo, cbi, 3, 3, P, Mo), np.float32)
        for m in range(cbo):
            for cb in range(cbi):
                for ky in range(3):
                    for kx in range(3):
                        blob[m, cb, ky, kx] = uw[cb * P:(cb + 1) * P,
                                                 m * Mo:(m + 1) * Mo, ky, kx]
        w[f"wu{i}"] = blob

    # final conv 7x7 64->3, stage A: K = j*64+ci, M = r*21 + dx*3 + co;
    # round t reads padded rows y0+2t+j => ky = 2t + j - r.
    wf = np.asarray(inp["wf"], np.float32)  # [3, 64, 7, 7]
    wfA = np.zeros((6, 128, 126), np.float32)
    for t in range(6):
        for j in range(2):
            for r in range(6):
                ky = 2 * t + j - r
                if 0 <= ky <= 6:
                    for dx in range(7):
                        for co in range(3):
                            wfA[t, j * 64:(j + 1) * 64, r * 21 + dx * 3 + co] = \
                                wf[co, :, ky, dx]
    wfS = np.zeros((7, 126, 18), np.float32)
    for dx in range(7):
        for r in range(6):
            for co in range(3):
                wfS[dx, r * 21 + dx * 3 + co, r * 3 + co] = 1.0
    w["wfA"] = wfA
    w["wfS"] = wfS
    bf = np.asarray(inp["bf"], np.float32)
    w["bfv"] = np.tile(bf, 6).reshape(18, 1).astype(np.float32)

    # L1 stats fold: average the (r=0, co) and (r=1, co) partition pairs.
    wfold = np.zeros((128, 64), np.float32)
    for r in range(2):
        for co in range(64):
            wfold[r * 64 + co, co] = 0.5
    w["wfold"] = wfold
    return w


def prep_core_inputs(x_img, inst_img, wblobs):
    xpad = np.pad(np.asarray(x_img, np.float32), ((0, 0), (3, 3), (3, 3)),
                  mode="reflect")  # [3, 518, 518]
    xrep = np.zeros((48, 512, 518), np.float32)
    for ci in range(3):
        for dy in range(8):
            for l in range(2):
                p = ci * 16 + dy * 2 + l
                rmax = min(512, 518 - dy)
                cmax = 518 - l
                xrep[p, :rmax, :cmax] = xpad[ci, dy:dy + rmax, l:l + cmax]
    instp = (np.asarray(inst_img) == 1).reshape(P, 2048).astype(np.float32)
    m = {"xrep": xrep, "instp": instp}
    m.update(wblobs)
    return m


# ----------------------------------------------------------------------------
# Device kernel
# ----------------------------------------------------------------------------

def build_kernel(debug=False):
    nc = bacc.Bacc(None, target_bir_lowering=False)

    xrep = nc.dram_tensor("xrep", [48, 512, 518], F32, kind="ExternalInput")
    instp = nc.dram_tensor("instp", [P, 2048], F32, kind="ExternalInput")
    w1 = nc.dram_tensor("w1", [4, 48, 128], F32, kind="ExternalInput")
    wd, wu = [], []
    c = BASE
    for i in range(NL):
        wd.append(nc.dram_tensor(
            f"wd{i}", [(2 * c) // P or 1, max(c // P, 1), 3, 3, min(c, P), P],
            F32, kind="ExternalInput"))
        c *= 2
    for i in range(NL):
        wu.append(nc.dram_tensor(
            f"wu{i}", [max((c // 2) // P, 1), c // P, 3, 3, P, min(c // 2, P)],
            F32, kind="ExternalInput"))
        c //= 2
    wfA = nc.dram_tensor("wfA", [6, 128, 126], F32, kind="ExternalInput")
    wfS = nc.dram_tensor("wfS", [7, 126, 18], F32, kind="ExternalInput")
    bfv = nc.dram_tensor("bfv", [18, 1], F32, kind="ExternalInput")
    wfold = nc.dram_tensor("wfold", [128, 64], F32, kind="ExternalInput")

    h0 = nc.dram_tensor("h0", [64, 512, 512], F32)
    h1 = nc.dram_tensor("h1", [1, P, 256, 256], F32)
    h2 = nc.dram_tensor("h2", [2, P, 128, 128], F32)
    h3 = nc.dram_tensor("h3", [4, P, 64, 64], F32)
    h4 = nc.dram_tensor("h4", [8, P, 32, 32], F32)
    g0 = nc.dram_tensor("g0", [4, P, 64, 64], F32)
    g1 = nc.dram_tensor("g1", [2, P, 128, 128], F32)
    g2 = nc.dram_tensor("g2", [1, P, 256, 256], F32)
    g3 = nc.dram_tensor("g3", [64, 518, 518], F32)  # padded coords
    hf = nc.dram_tensor("hf", [3, 512, 512], F32)
    scal = nc.dram_tensor("scal", [4, 1], F32)
    out = nc.dram_tensor("out", [3, 512, 512], F32, kind="ExternalOutput")

    with tile.TileContext(nc) as tc, contextlib.ExitStack() as ctx:
        sb = ctx.enter_context(tc.tile_pool(name="sb", bufs=2))
        wsm = ctx.enter_context(tc.tile_pool(name="wsm", bufs=1))
        nrm = ctx.enter_context(tc.tile_pool(name="nrm", bufs=1))
        stp = ctx.enter_context(tc.tile_pool(name="stp", bufs=1))
        ps = ctx.enter_context(tc.tile_pool(name="ps", bufs=4, space="PSUM"))
        psf = ctx.enter_context(tc.tile_pool(name="psf", bufs=1, space="PSUM"))

        eps_t = nrm.tile([P, 1], F32, name="eps_t")
        nc.vector.memset(eps_t, EPS)
        ones_t = nrm.tile([P, 1], F32, name="ones_t")
        nc.vector.memset(ones_t, 1.0)

        def finalize_stats(mv_ap, npart, name):
            """mv [npart, 2] (mean, var) -> s_t [npart, 2] = (scale, bias)."""
            s_t = nrm.tile([npart, 2], F32, name=f"st_{name}", tag=f"st_{name}")
            tmp = stp.tile([npart, 1], F32, name=f"tmp_{name}", tag="stmp")
            nc.scalar.activation(out=tmp, in_=mv_ap[:, 1:2], func=AF.Sqrt,
                                 bias=eps_t[:npart], scale=1.0)
            nc.vector.reciprocal(out=s_t[:, 0:1], in_=tmp)
            nc.vector.tensor_scalar(out=s_t[:, 1:2], in0=mv_ap[:, 0:1],
                                    scalar1=s_t[:, 0:1], scalar2=-1.0,
                                    op0=ALU.mult, op1=ALU.mult)
            return s_t

        # ================= layer 1: 7x7 conv, 3 -> 64 ======================
        _sc = nc.enter_named_scope("L1", False)[0]
        w1t = wsm.tile([48, 4, 128], F32R, name="w1t")
        nc.sync.dma_start(out=w1t, in_=w1[:, :, :]
                          .rearrange("d k m -> k d m").bitcast(F32R))
        NS1, NP1 = 64, 4  # strips x row-pairs per strip
        st1 = stp.tile([P, NS1 * NP1, 6], F32, name="st1", tag="stats")
        for s_i in range(NS1):
            y0 = s_i * 8
            slab = sb.tile([48, 4, 518], F32R, name="slab1", tag="inslab")
            nc.sync.dma_start(out=slab,
                              in_=_ap(xrep[0:48, 0, 0], y0 * 518,
                                      [[2 * 518, 4], [1, 518]]).bitcast(F32R))
            oslab = sb.tile([P, 4, 512], F32, name="oslab1", tag="outslab")
            for k in range(NP1):
                pt = ps.tile([P, 512], F32, name="pt1", tag="mm")
                for d in range(4):
                    rhs = _ap(slab[:, 0, 0], k * 518 + 2 * d, [[1, 512]])
                    nc.tensor.matmul(pt, w1t[:, d, :], rhs,
                                     start=(d == 0), stop=(d == 3))
                nc.vector.bn_stats(out=st1[:, s_i * 4 + k, :], in_=pt)
                nc.scalar.activation(out=oslab[:, k, :], in_=pt, func=AF.Copy)
            nc.sync.dma_start(
                out=_ap(h0[0:64, 0, 0], y0 * 512, [[2 * 512, 4], [1, 512]]),
                in_=oslab[0:64, :, :])
            nc.sync.dma_start(
                out=_ap(h0[0:64, 0, 0], (y0 + 1) * 512, [[2 * 512, 4], [1, 512]]),
                in_=oslab[64:128, :, :])
        mv1 = stp.tile([P, 2], F32, name="mv1", tag="mv")
        nc.vector.bn_aggr(out=mv1, in_=st1)
        me1 = stp.tile([P, 2], F32, name="me1", tag="me")
        nc.vector.tensor_copy(out=me1[:, 0:1], in_=mv1[:, 0:1])
        nc.vector.tensor_scalar(out=me1[:, 1:2], in0=mv1[:, 0:1],
                                scalar1=mv1[:, 0:1], scalar2=None, op0=ALU.mult)
        nc.vector.tensor_add(out=me1[:, 1:2], in0=me1[:, 1:2], in1=mv1[:, 1:2])
        wft = wsm.tile([P, 64], F32, name="wft")
        nc.sync.dma_start(out=wft, in_=wfold[:, :])
        pm = ps.tile([64, 2], F32, name="pm", tag="mini", bufs=1)
        nc.tensor.matmul(pm, wft, me1, start=True, stop=True)
        # fold output pm = [64, (mean, E[x^2])]; var = E[x^2] - mean^2
        msq = stp.tile([64, 1], F32, name="msq", tag="msq")
        mv0g = stp.tile([64, 2], F32, name="mv0g", tag="mvg")
        nc.scalar.activation(out=mv0g, in_=pm, func=AF.Copy)
        nc.vector.tensor_scalar(out=msq, in0=mv0g[:, 0:1],
                                scalar1=mv0g[:, 0:1], scalar2=None, op0=ALU.mult)
        nc.vector.tensor_scalar(out=msq, in0=msq, scalar1=-1.0, scalar2=None,
                                op0=ALU.mult)
        nc.vector.tensor_add(out=mv0g[:, 1:2], in0=mv0g[:, 1:2], in1=msq)
        st_h0 = finalize_stats(mv0g, 64, "h0")

        # ================= down convs ======================================
        def down_layer(li, src, dst, wsrc, st_in, Cin, Cout, Hi, nr, nrc):
            Wi = Hi
            Ho, Wo = Hi // 2, Wi // 2
            cbi, cbo, K = max(Cin // P, 1), Cout // P, min(Cin, P)
            Wp = Wi + 2
            nstrip, nchunk = Ho // nr, nr // nrc
            rows_in = 2 * nr + 1
            stt = stp.tile([P, cbo, nstrip * nchunk, 6], F32,
                           name=f"std{li}", tag="stats")
            for s_i in range(nstrip):
                y0 = s_i * nr
                i0 = 2 * y0 - 1
                slab = sb.tile([K, cbi, rows_in, Wp], F32R,
                               name=f"sld{li}", tag="inslab")
                lo, hi = max(i0, 0), min(i0 + rows_in, Hi)
                nc.gpsimd.memset(slab.bitcast(F32)[:, :, :, 0:1], 0.0)
                nc.gpsimd.memset(slab.bitcast(F32)[:, :, :, Wi + 1:Wp], 0.0)
                if lo > i0:
                    nc.gpsimd.memset(slab.bitcast(F32)[:, :, 0:lo - i0, :], 0.0)
                if hi < i0 + rows_in:
                    nc.gpsimd.memset(
                        slab.bitcast(F32)[:, :, hi - i0:rows_in, :], 0.0)
                for cb in range(cbi):
                    if src is h0:
                        sap = _ap(src[0:K, 0, 0], lo * Wi,
                                  [[Wi, hi - lo], [1, Wi]])
                    else:
                        sap = _ap(src[cb, 0:K, 0, 0], lo * Wi,
                                  [[Wi, hi - lo], [1, Wi]])
                    nc.sync.dma_start(out=slab[:, cb, lo - i0:hi - i0, 1:Wi + 1],
                                      in_=sap.bitcast(F32R))
                    nc.scalar.activation(
                        out=slab[:, cb, lo - i0:hi - i0, 1:Wi + 1],
                        in_=slab[:, cb, lo - i0:hi - i0, 1:Wi + 1],
                        func=AF.Relu, bias=st_in[cb][:, 1:2],
                        scale=st_in[cb][:, 0:1])
                oslab = sb.tile([P, cbo, nr, Wo], F32, name=f"osd{li}",
                                tag="outslab")
                for m in range(cbo):
                    wt = sb.tile([K, cbi, 3, 3, P], F32R, name=f"wtd{li}",
                                 tag="w")
                    nc.sync.dma_start(
                        out=wt, in_=wsrc[m, :, :, :, :, :]
                        .rearrange("cb dy dx k m2 -> k cb dy dx m2")
                        .bitcast(F32R))
                    for ch in range(nchunk):
                        pt = ps.tile([P, nrc, Wo], F32, name=f"ptd{li}",
                                     tag="mm")
                        first = True
                        for cb in range(cbi):
                            for dy in range(3):
                                for dx in range(3):
                                    row0 = 2 * (y0 + ch * nrc) - 1 + dy - i0
                                    rhs = _ap(slab[:, 0, 0, 0],
                                              cb * rows_in * Wp + row0 * Wp + dx,
                                              [[2 * Wp, nrc], [2, Wo]])
                                    last = (cb == cbi - 1 and dy == 2
                                            and dx == 2)
                                    nc.tensor.matmul(pt, wt[:, cb, dy, dx, :],
                                                     rhs, start=first,
                                                     stop=last)
                                    first = False
                        nc.vector.bn_stats(out=stt[:, m, s_i * nchunk + ch, :],
                                           in_=pt.rearrange("p a b -> p (a b)"))
                        nc.scalar.activation(
                            out=oslab[:, m, ch * nrc:(ch + 1) * nrc, :],
                            in_=pt, func=AF.Copy)
                for m in range(cbo):
                    nc.sync.dma_start(
                        out=_ap(dst[m, 0:P, 0, 0], y0 * Wo,
                                [[Wo, nr], [1, Wo]]),
                        in_=oslab[:, m, :, :])
            st_outs = []
            for m in range(cbo):
                mv = stp.tile([P, 2], F32, name=f"mvd{li}", tag="mv")
                nc.vector.bn_aggr(out=mv, in_=stt[:, m, :, :])
                st_outs.append(finalize_stats(mv, P, f"d{li}m{m}"))
            return st_outs

        nc.leave_named_scope("L1", _sc, False)
        _sc = nc.enter_named_scope("down", False)[0]
        st_h1 = down_layer(0, h0, h1, wd[0], [st_h0], 64, 128, 512,
                           nr=4, nrc=2)
        st_h2 = down_layer(1, h1, h2, wd[1], st_h1, 128, 256, 256,
                           nr=8, nrc=4)
        st_h3 = down_layer(2, h2, h3, wd[2], st_h2, 256, 512, 128,
                           nr=8, nrc=8)
        st_h4 = down_layer(3, h3, h4, wd[3], st_h3, 512, 1024, 64,
                           nr=8, nrc=8)

        # ================= up convs ========================================
        def up_layer(li, src, dst, wsrc, st_in, Cin, Cout, Hi, nr):
            Wi = Hi
            Ho, Wo = 2 * Hi, 2 * Wi
            cbi, cbo, Mo = Cin // P, max(Cout // P, 1), min(Cout, P)
            Wp = Wi + 1
            nstrip = Ho // nr
            maxch = nstrip * max(8 * (nr // 2 * Wi // 512), 4)
            stt = stp.tile([P, cbo, maxch, 6], F32, name=f"stu{li}",
                           tag="stats")
            nch = [0] * cbo
            for m in range(cbo):
                wt = sb.tile([P, cbi, 3, 3, Mo], F32R, name=f"wtu{li}",
                             tag="wup", bufs=1)
                nc.sync.dma_start(
                    out=wt, in_=wsrc[m, :, :, :, :, :]
                    .rearrange("cb ky kx k m2 -> k cb ky kx m2").bitcast(F32R))
                for s_i in range(nstrip):
                    y0 = s_i * nr
                    i_lo = max((y0 - 1) // 2, 0)
                    i_hi = min((y0 + nr) // 2 + 1, Hi)
                    rows_in = i_hi - i_lo + 1  # + end gutter row
                    slab = sb.tile([P, cbi, rows_in, Wp], F32R,
                                   name=f"slu{li}", tag="inslab")
                    nc.gpsimd.memset(slab.bitcast(F32)[:, :, :, Wi:Wp], 0.0)
                    nc.gpsimd.memset(
                        slab.bitcast(F32)[:, :, rows_in - 1:rows_in, :], 0.0)
                    for cb in range(cbi):
                        nc.sync.dma_start(
                            out=slab[:, cb, 0:i_hi - i_lo, 0:Wi],
                            in_=_ap(src[cb, 0:P, 0, 0], i_lo * Wi,
                                    [[Wi, i_hi - i_lo], [1, Wi]])
                            .bitcast(F32R))
                        nc.scalar.activation(
                            out=slab[:, cb, 0:i_hi - i_lo, 0:Wi],
                            in_=slab[:, cb, 0:i_hi - i_lo, 0:Wi],
                            func=AF.Relu, bias=st_in[cb][:, 1:2],
                            scale=st_in[cb][:, 0:1])
                    oslab = sb.tile([Mo, nr, Wo], F32, name=f"osu{li}",
                                    tag="outslab")
                    for a in range(2):
                        kys = [1] if a == 0 else [0, 2]
                        for b in range(2):
                            kxs = [1] if b == 0 else [0, 2]
                            n_cr = nr // 2
                            nsub = max(n_cr * Wi // 512, 1)
                            rsub = n_cr // nsub
                            for su in range(nsub):
                                yb = y0 + a + 2 * su * rsub
                                pt = ps.tile([Mo, rsub, Wi], F32,
                                             name=f"ptu{li}", tag="mm")
                                first = True
                                for cb in range(cbi):
                                    for ky in kys:
                                        i_first = (yb + 1 - ky) // 2
                                        for kx in kxs:
                                            j0 = (b + 1 - kx) // 2
                                            rhs = _ap(
                                                slab[:, 0, 0, 0],
                                                cb * rows_in * Wp
                                                + (i_first - i_lo) * Wp + j0,
                                                [[Wp, rsub], [1, Wi]])
                                            last = (cb == cbi - 1
                                                    and ky == kys[-1]
                                                    and kx == kxs[-1])
                                            nc.tensor.matmul(
                                                pt, wt[:, cb, ky, kx, :], rhs,
                                                start=first, stop=last)
                                            first = False
                                nc.vector.bn_stats(
                                    out=stt[:Mo, m, nch[m], :],
                                    in_=pt.rearrange("p a b -> p (a b)"))
                                nch[m] += 1
                                oap = _ap(oslab[:, 0, 0],
                                          (a + 2 * su * rsub) * Wo + b,
                                          [[2 * Wo, rsub], [2, Wi]])
                                nc.scalar.activation(out=oap, in_=pt,
                                                     func=AF.Copy)
                    if dst is g3:
                        nc.sync.dma_start(
                            out=_ap(g3[0:64, 0, 0], (3 + y0) * 518 + 3,
                                    [[518, nr], [1, Wo]]),
                            in_=oslab[:, :, :])
                        # column reflect gutters (padded cols 0..2, 515..517)
                        for cc in range(3):
                            nc.sync.dma_start(
                                out=_ap(g3[0:64, 0, 0], (3 + y0) * 518 + cc,
                                        [[518, nr], [1, 1]]),
                                in_=oslab[:, :, 3 - cc:4 - cc])
                            nc.sync.dma_start(
                                out=_ap(g3[0:64, 0, 0],
                                        (3 + y0) * 518 + 515 + cc,
                                        [[518, nr], [1, 1]]),
                                in_=oslab[:, :, 510 - cc:511 - cc])
                    else:
                        nc.sync.dma_start(
                            out=_ap(dst[m, 0:P, 0, 0], y0 * Wo,
                                    [[Wo, nr], [1, Wo]]),
                            in_=oslab[:, :, :])
            st_outs = []
            for m in range(cbo):
                mv = stp.tile([Mo, 2], F32, name=f"mvu{li}", tag="mv")
                nc.vector.bn_aggr(out=mv, in_=stt[:Mo, m, 0:nch[m], :])
                st_outs.append(finalize_stats(mv, Mo, f"u{li}m{m}"))
            return st_outs

        nc.leave_named_scope("down", _sc, False)
        _sc = nc.enter_named_scope("up", False)[0]
        st_g0 = up_layer(0, h4, g0, wu[0], st_h4, 1024, 512, 32, nr=16)
        st_g1 = up_layer(1, g0, g1, wu[1], st_g0, 512, 256, 64, nr=32)
        st_g2 = up_layer(2, g1, g2, wu[2], st_g1, 256, 128, 128, nr=32)
        st_g3 = up_layer(3, g2, g3, wu[3], st_g2, 128, 64, 256, nr=8)

        # g3 row reflect gutters (padded rows 0..2 and 515..517)
        for r_ in range(3):
            nc.sync.dma_start(out=_ap(g3[0:64, 0, 0], r_ * 518, [[1, 518]]),
                              in_=_ap(g3[0:64, 0, 0], (6 - r_) * 518,
                                      [[1, 518]]))
            nc.sync.dma_start(out=_ap(g3[0:64, 0, 0], (515 + r_) * 518,
                                      [[1, 518]]),
                              in_=_ap(g3[0:64, 0, 0], (513 - r_) * 518,
                                      [[1, 518]]))

        # ================= final conv 7x7, 64 -> 3, tanh ==================
        nc.leave_named_scope("up", _sc, False)
        _sc = nc.enter_named_scope("final", False)[0]
        sF = nrm.tile([P, 2], F32, name="sF")
        nc.sync.dma_start(out=sF[0:64, :], in_=st_g3[0][:, :])
        nc.sync.dma_start(out=sF[64:128, :], in_=st_g3[0][:, :])
        wfAt = wsm.tile([P, 6, 126], F32R, name="wfAt")
        nc.sync.dma_start(out=wfAt, in_=wfA[:, :, :]
                          .rearrange("t k m -> k t m").bitcast(F32R))
        wfSt = wsm.tile([126, 7, 18], F32R, name="wfSt")
        nc.sync.dma_start(out=wfSt, in_=wfS[:, :, :]
                          .rearrange("d k m -> k d m").bitcast(F32R))
        bft = wsm.tile([18, 1], F32, name="bft")
        nc.sync.dma_start(out=bft, in_=bfv[:, :])

        for y0 in list(range(0, 505, 6)) + [506]:
            slab = sb.tile([P, 6, 518], F32R, name="slF", tag="inslab")
            for j in range(2):
                nc.sync.dma_start(
                    out=slab[j * 64:(j + 1) * 64, :, :],
                    in_=_ap(g3[0:64, 0, 0], (y0 + j) * 518,
                            [[2 * 518, 6], [1, 518]]).bitcast(F32R))
            nc.scalar.activation(out=slab, in_=slab, func=AF.Relu,
                                 bias=sF[:, 1:2], scale=sF[:, 0:1])
            for hx in range(2):
                ptA = psf.tile([126, 262], F32, name="ptA", tag="fa", bufs=2)
                for t in range(6):
                    rhs = _ap(slab[:, 0, 0], t * 518 + hx * 256, [[1, 262]])
                    nc.tensor.matmul(ptA, wfAt[:, t, :], rhs,
                                     start=(t == 0), stop=(t == 5))
                stg = sb.tile([126, 262], F32R, name="stg", tag="outslab")
                nc.scalar.activation(out=stg, in_=ptA, func=AF.Copy)
                ptB = psf.tile([18, 256], F32, name="ptB", tag="fb", bufs=1)
                for dx in range(7):
                    nc.tensor.matmul(ptB, wfSt[:, dx, :], stg[:, dx:dx + 256],
                                     start=(dx == 0), stop=(dx == 6))
                ftile = sb.tile([18, 256], F32, name="ftile", tag="ftile")
                nc.scalar.activation(out=ftile, in_=ptB, func=AF.Tanh,
                                     bias=bft, scale=1.0)
                nc.sync.dma_start(
                    out=_dap(hf, y0 * 512 + hx * 256,
                             [[512, 6], [512 * 512, 3], [1, 256]]),
                    in_=ftile)

        # ================= masked segment mean ============================
        nc.leave_named_scope("final", _sc, False)
        mask = sb.tile([P, 2048], F32, name="mask", tag="w")
        nc.sync.dma_start(out=mask, in_=instp[:, :])
        G = nrm.tile([P, 4], F32, name="G", tag="segG")
        for c_ in range(3):
            hc = sb.tile([P, 2048], F32, name="hc", tag="inslab")
            nc.sync.dma_start(out=hc,
                              in_=_dap(hf, c_ * 512 * 512, [[2048, P], [1, 2048]]))
            pc = sb.tile([P, 2048], F32, name="pc", tag="outslab")
            nc.vector.tensor_mul(out=pc, in0=hc, in1=mask)
            nc.vector.tensor_reduce(out=G[:, c_:c_ + 1], in_=pc, op=ALU.add,
                                    axis=mybir.AxisListType.X)
        nc.vector.tensor_reduce(out=G[:, 3:4], in_=mask, op=ALU.add,
                                axis=mybir.AxisListType.X)
        if debug:
            dbg_mask = nc.dram_tensor("dbg_mask", [P, 2048], F32,
                                      kind="ExternalOutput")
            nc.sync.dma_start(out=dbg_mask[:, :], in_=mask)
            dbg_G = nc.dram_tensor("dbg_G", [P, 4], F32, kind="ExternalOutput")
            nc.sync.dma_start(out=dbg_G[:, :], in_=G)
        pg = ps.tile([4, 1], F32, name="pg", tag="mini", bufs=1)
        nc.tensor.matmul(pg, G, ones_t, start=True, stop=True)
        g4 = nrm.tile([4, 1], F32, name="g4", tag="seg4")
        nc.scalar.activation(out=g4, in_=pg, func=AF.Copy)
        nc.sync.dma_start(out=scal[:, :], in_=g4)
        gb = nrm.tile([P, 4], F32, name="gb", tag="segb")
        nc.gpsimd.dma_start(out=gb, in_=_dap(scal, 0, [[0, P], [1, 4]]))
        if debug:
            dbg_gb = nc.dram_tensor("dbg_gb", [P, 4], F32, kind="ExternalOutput")
            nc.sync.dma_start(out=dbg_gb[:, :], in_=gb)
        rcp = nrm.tile([P, 1], F32, name="rcp", tag="segr")
        nc.vector.reciprocal(out=rcp, in_=gb[:, 3:4])
        for c_ in range(3):
            mc = nrm.tile([P, 1], F32, name="mc", tag="segm")
            nc.vector.tensor_mul(out=mc, in0=gb[:, c_:c_ + 1], in1=rcp)
            oc = sb.tile([P, 2048], F32, name="oc", tag="outslab")
            nc.vector.tensor_scalar(out=oc, in0=mask, scalar1=mc,
                                    scalar2=None, op0=ALU.mult)
            nc.sync.dma_start(out=_dap(out, c_ * 512 * 512,
                                       [[2048, P], [1, 2048]]),
                              in_=oc)

        if debug:
            for nm, tens, sh in [
                ("h0", h0, [64, 262144]), ("h1", h1, [128, 65536]),
                ("h2", h2, [256, 16384]), ("h3", h3, [512, 4096]),
                ("h4", h4, [1024, 1024]), ("g0", g0, [512, 4096]),
                ("g1", g1, [256, 16384]), ("g2", g2, [128, 65536]),
                ("g3", g3, [64, 268324]), ("hf", hf, [3, 262144]),
            ]:
                dbg = nc.dram_tensor("dbg_" + nm, sh, F32,
                                     kind="ExternalOutput")
                nc.sync.dma_start(
                    out=_dap(dbg, 0, [[sh[1], sh[0]], [1, sh[1]]]),
                    in_=_dap(tens, 0, [[sh[1], sh[0]], [1, sh[1]]]))

    nc.finalize()
    return nc


_CACHE = {}


def kernel(**inputs):
    if "nc" not in _CACHE:
        _CACHE["nc"] = build_kernel()
    nc = _CACHE["nc"]
    wblobs = prep_weights(inputs)
    x = np.asarray(inputs["x"], np.float32)
    inst = np.asarray(inputs["inst"])
    in_maps = [prep_core_inputs(x[c % B], inst[c % B, 0], wblobs)
               for c in range(8)]
    res = run_bass_kernel_spmd(nc, in_maps, core_ids=list(range(8)))
    return np.stack([res.results[c]["out"] for c in range(B)], 0)

